# revision 27
# baseline (speedup 1.0000x reference)
"""Trainium2 Bass kernel for the DQN hypergraph-conv network (8-core SPMD).

Sharding: edges row-sharded for the message stage (Hs@X@theta local per
edge shard), nodes column-sharded for the aggregation stage (Ht.T @ ...),
with AllGather collectives moving the small [E,H]/[N,H] intermediates.
The big Ht/Hs shards are read once in bf16 and stay resident in SBUF
across both conv layers.

Per core c (NCORES=8):
  hsT = Hs[e_c, :].T   [N, E/8]  bf16   (stage-1 moving operand)
  ht  = Ht[:, n_c]     [E, N/8]  bf16   (stage-2 moving operand)
  stage1: tmpT[f,e] = sum_n X[n-tile].T @ hsT[n-tile]      (PE, N=512 free)
  msg[e,h] = tmpT.T @ theta ; scaled = edge_w * msg        -> AllGather
  stage2: aggT[h,n] = w_trans.T @ xiT + bias (rank-1)
          + sum_e scaled[e-tile].T @ ht[e-tile]            (PE, N=512 free)
  epilogue: fused leaky-relu / dropout mask / second lrelu  (DVE)
  conv0 only: PE-transpose X1T -> X1 tiles -> AllGather for conv1
  fc: fc_w.T @ XT (f32 matmul) + host-precomputed state term -> [1, N/8]

Performance notes:
 - Every dma_start costs ~0.6-2us of serial issue time on its engine and
   one InstDMACopy already spans all 16 SDMA engines, so transfers are
   consolidated into a few large DMAs spread over three trigger rings
   (sync, scalar, vector).
 - The big operands are packed TILE-MAJOR on the host ([128, nblk, row])
   so each DMA partition-row is an 8-32KB contiguous read (full HBM BW;
   the natural [N, 128]-style layouts only give 256B rows).
 - Loads are chained with explicit deps so chunks arrive pipelined in
   consumption order instead of fair-sharing bandwidth and all finishing
   together; ht is deferred behind stage 1 to fill the AllGather window.
"""

import sys

if "/opt/trn_rl_repo" not in sys.path:
    sys.path.insert(0, "/opt/trn_rl_repo")

import numpy as np
import ml_dtypes

NCORES = 8
N, E, F = 8192, 4096, 128
E_SH = E // NCORES   # 512 edges per core
N_SH = N // NCORES   # 1024 nodes per core
NEG_SLOPE = 0.01
DROP_P = 0.5

# packed bf16 params layout (columns)
PB_TH = 0          # th0, th1         [128, 128] each
PB_WT = 256        # wt0, wt1
PB_IDN = 512       # identity
PB_XIT = 640       # xiT              [128, 1024]
PB_B = 1664        # b0, b1 on partition 0, 128 cols each
PB_W = 1920
# packed f32 params layout (columns)
PF_EW = 0          # ew0, ew1         [128, 4] each
PF_M2T = 8         # mask2T           [128, 1024]
PF_FCW = 1032      # fc_w[:128]       [128, 1]
PF_ST = 1033       # state term on partition 0, 1024 cols
PF_W = 2060

_CACHE = {}


def _build_nc():
    import concourse.bacc as bacc
    import concourse.mybir as mybir
    import concourse.tile as tile
    from concourse.tile import add_dep_helper

    bf16 = mybir.dt.bfloat16
    f32 = mybir.dt.float32
    Alu = mybir.AluOpType

    nc = bacc.Bacc("TRN2", target_bir_lowering=False, debug=False,
                   num_devices=NCORES)

    hsT_d = nc.dram_tensor("hsT", [128, 64, E_SH], bf16, kind="ExternalInput")
    ht_d = nc.dram_tensor("ht", [128, 32, N_SH], bf16, kind="ExternalInput")
    xbf_d = nc.dram_tensor("xbf", [128, 64, F], bf16, kind="ExternalInput")
    pbf_d = nc.dram_tensor("pbf", [128, PB_W], bf16, kind="ExternalInput")
    pf32_d = nc.dram_tensor("pf32", [128, PF_W], f32, kind="ExternalInput")
    out_d = nc.dram_tensor("out", [1, N_SH], f32, kind="ExternalOutput")

    RG = [list(range(NCORES))]

    with tile.TileContext(nc) as tc:
        with (
            tc.tile_pool(name="sb", bufs=1) as sb,
            tc.tile_pool(name="sc2", bufs=2) as sc2,
            tc.tile_pool(name="ps_tmp", bufs=1, space="PSUM") as ps_tmp,
            tc.tile_pool(name="ps_agg", bufs=2, space="PSUM") as ps_agg,
            tc.tile_pool(name="ps_sm", bufs=2, space="PSUM") as ps_sm,
            tc.tile_pool(name="dram", bufs=1, space="DRAM") as dram,
        ):
            # ---- warmup collective ----
            # The first collective of a NEFF pays the NRT global barrier +
            # ncfw setup (~15-25us).  Run a tiny AllGather with no deps so
            # that cost lands in the load shadow, not on the msg0 path.
            warm_sb = sb.tile([1, 64], bf16, tag="warm_sb")
            nc.gpsimd.memset(warm_sb[:], 0.0)
            warm_in = dram.tile([1, 64], bf16, tag="warm_in")
            warm_out = dram.tile([NCORES, 64], bf16, addr_space="Shared",
                                 tag="warm_out")
            nc.gpsimd.dma_start(warm_in[:], warm_sb[:])
            nc.gpsimd.collective_compute(
                "AllGather", mybir.AluOpType.bypass,
                replica_groups=[list(range(NCORES))],
                ins=[warm_in[:]], outs=[warm_out[:]])

            # ---- two pipelined load chains ----
            # sync ring:   hsT chunks 0..3, serially chained so each runs at
            #              full single-DMA bandwidth and arrives in the order
            #              stage 1 consumes it
            # scalar ring: x0, x1, pbf, pf32 (then ht after stage 1)
            hsT_t = [sb.tile([128, 16, E_SH], bf16, tag=f"hsT{i}",
                             name=f"hsT{i}") for i in range(4)]
            xall = [sb.tile([128, 32, F], bf16, tag=f"x{i}", name=f"x{i}")
                    for i in range(2)]
            pbf = sb.tile([128, PB_W], bf16, tag="pbf")
            pf = sb.tile([128, PF_W], f32, tag="pf")

            prev = {"sync": None, "vec": None, "sca": None}

            def chain(ring, engine, dst, src):
                dma = engine.dma_start(dst, src)
                if prev[ring] is not None:
                    add_dep_helper(dma.ins, prev[ring].ins, sync=True,
                                   reason="pipelined load chain")
                prev[ring] = dma
                return dma

            for k in range(4):
                chain("sync", nc.sync, hsT_t[k][:],
                      hsT_d[:, 16 * k:16 * (k + 1), :])
            chain("sca", nc.scalar, xall[0][:], xbf_d[:, 0:32, :])
            chain("sca", nc.scalar, xall[1][:], xbf_d[:, 32:64, :])
            chain("sca", nc.scalar, pbf[:], pbf_d[:, :])
            chain("sca", nc.scalar, pf[:], pf32_d[:, :])

            def th(conv):
                return pbf[:, PB_TH + conv * 128:PB_TH + (conv + 1) * 128]

            def wt(conv):
                return pbf[:, PB_WT + conv * 128:PB_WT + (conv + 1) * 128]

            idn = pbf[:, PB_IDN:PB_IDN + 128]
            xiT = pbf[:, PB_XIT:PB_XIT + 1024]

            def bias(conv):
                return pbf[0:1, PB_B + conv * 128:PB_B + (conv + 1) * 128]

            def ew(conv, ec):
                c0 = PF_EW + conv * 4 + ec
                return pf[:, c0:c0 + 1]

            m2T = pf[:, PF_M2T:PF_M2T + 1024]
            fcw = pf[:, PF_FCW:PF_FCW + 1]
            stT = pf[0:1, PF_ST:PF_ST + 1024]

            ones_sb = sb.tile([1, 512], bf16, tag="ones")
            nc.vector.memset(ones_sb[:], 1.0)

            ht_t = [None, None]

            # collective bounce buffers, PARTITION-MAJOR ([128, blk, F]
            # per rank) so the SBUF<->DRAM bounce DMAs on both sides are
            # contiguous multi-KB rows instead of 256B gathers
            agm_in = [dram.tile([128, 4, F], bf16, tag=f"agmi{i}",
                                name=f"agmi{i}") for i in range(2)]
            agm_out = [dram.tile([NCORES * 128, 4, F], bf16,
                                 addr_space="Shared",
                                 tag=f"agmo{i}", name=f"agmo{i}")
                       for i in range(2)]
            agx_in = dram.tile([128, 8, F], bf16, tag="agxi")
            agx_out = dram.tile([NCORES * 128, 8, F], bf16,
                                addr_space="Shared", tag="agxo")

            x1c = None           # gathered X1 for conv1 stage 1
            xT = [None, None]    # final-layer activations (f32)

            for conv in range(2):
                # ---------- stage 1: tmpT = X.T @ HsT ----------
                tmpT_ps = ps_tmp.tile([128, E_SH], f32, tag="tmpT")
                mm_last = None
                for nt in range(64):
                    lhsT = (xall[nt // 32][:, nt % 32, :] if conv == 0
                            else x1c[:, nt // 8, nt % 8, :])
                    mm_last = nc.tensor.matmul(
                        tmpT_ps[:], lhsT, hsT_t[nt // 16][:, nt % 16, :],
                        start=(nt == 0), stop=(nt == 63))

                if conv == 0:
                    # ht loads deferred behind stage 1: they fill the
                    # AllGather window instead of stealing stage-1 BW.
                    for i in range(2):
                        hc = sb.tile([128, 16, N_SH], bf16, tag=f"ht{i}",
                                     name=f"ht{i}")
                        dma = nc.scalar.dma_start(
                            hc[:], ht_d[:, i * 16:(i + 1) * 16, :])
                        add_dep_helper(dma.ins, mm_last.ins, sync=True,
                                       reason="defer ht behind stage1")
                        ht_t[i] = hc

                tmpT_bf = sb.tile([128, E_SH], bf16, tag=f"tmpTbf{conv}")
                nc.vector.tensor_copy(tmpT_bf[:], tmpT_ps[:])

                # ---------- msg = tmpT.T @ theta, scaled by edge_w ----------
                msg_sb = sb.tile([128, 4, F], bf16, tag="msg")
                for ec in range(4):
                    mps = ps_sm.tile([128, F], f32, tag="msg", bufs=2)
                    nc.tensor.matmul(
                        mps[:], tmpT_bf[:, ec * 128:(ec + 1) * 128],
                        th(conv), start=True, stop=True)
                    nc.vector.tensor_scalar(
                        msg_sb[:, ec, :], mps[:], ew(conv, ec), None, Alu.mult)
                nc.sync.dma_start(agm_in[conv][:], msg_sb[:])

                nc.gpsimd.collective_compute(
                    "AllGather", Alu.bypass, replica_groups=RG,
                    ins=[agm_in[conv][:]], outs=[agm_out[conv][:]])

                # [rank, p, c, h] -> sbuf [p, rank, c, h]; global edge tile
                # et = rank*4 + c
                sc_t = sb.tile([128, NCORES, 4, F], bf16, tag="sc")
                nc.scalar.dma_start(
                    sc_t[:], agm_out[conv].rearrange("(r p) c h -> p r c h",
                                                     p=128))

                # ---------- stage 2: aggT = wT@xiT + b + scaled.T @ Ht ----------
                for nb in range(2):
                    agg = ps_agg.tile([128, 512], f32, tag="agg")
                    nc.tensor.matmul(
                        agg[:], wt(conv), xiT[:, nb * 512:(nb + 1) * 512],
                        start=True, stop=False)
                    nc.tensor.matmul(
                        agg[:], bias(conv), ones_sb[:],
                        start=False, stop=False)
                    for et in range(32):
                        nc.tensor.matmul(
                            agg[:], sc_t[:, et // 4, et % 4, :],
                            ht_t[et // 16][:, et % 16,
                                           nb * 512:(nb + 1) * 512],
                            start=False, stop=(et == 31))

                    if conv == 0:
                        # X1T = lrelu(agg) * dropout_mask, transpose to
                        # node-major, bounce out (per nb, pipelined)
                        sl = sc2.tile([128, 512], f32, tag="sl")
                        nc.vector.tensor_scalar(
                            sl[:], agg[:], NEG_SLOPE, None, Alu.mult)
                        lr = sc2.tile([128, 512], f32, tag="lr")
                        nc.vector.tensor_tensor(lr[:], agg[:], sl[:], Alu.max)
                        x1t = sb.tile([128, 512], bf16, tag=f"x1t{nb}")
                        nc.vector.tensor_tensor(
                            x1t[:], lr[:], m2T[:, nb * 512:(nb + 1) * 512],
                            Alu.mult)

                        x1loc = sb.tile([128, 4, F], bf16, tag=f"x1loc{nb}",
                                        name=f"x1loc{nb}")
                        for t in range(4):
                            tps = ps_sm.tile([128, 128], bf16, tag="tr",
                                             bufs=2)
                            nc.tensor.transpose(
                                tps[:], x1t[:, t * 128:(t + 1) * 128], idn)
                            nc.vector.tensor_copy(x1loc[:, t, :], tps[:])
                        nc.sync.dma_start(
                            agx_in[:, nb * 4:(nb + 1) * 4, :], x1loc[:])
                    else:
                        # X = lrelu(lrelu(agg)) = max(agg, 1e-4*agg)  (f32)
                        sl = sc2.tile([128, 512], f32, tag="sl")
                        nc.vector.tensor_scalar(
                            sl[:], agg[:], NEG_SLOPE * NEG_SLOPE, None,
                            Alu.mult)
                        t = sb.tile([128, 512], f32, tag=f"xT{nb}")
                        nc.vector.tensor_tensor(t[:], agg[:], sl[:], Alu.max)
                        xT[nb] = t
                        # fc for this block immediately
                        fps = ps_sm.tile([1, 512], f32, tag="fc", bufs=1)
                        nc.tensor.matmul(fps[:], fcw, t[:],
                                         start=True, stop=True)
                        osb = sc2.tile([1, 512], f32, tag="osb")
                        nc.vector.tensor_tensor(
                            osb[:], fps[:], stT[:, nb * 512:(nb + 1) * 512],
                            Alu.add)
                        nc.sync.dma_start(
                            out_d[0:1, nb * 512:(nb + 1) * 512], osb[:])

                if conv == 0:
                    nc.gpsimd.collective_compute(
                        "AllGather", Alu.bypass, replica_groups=RG,
                        ins=[agx_in[:]], outs=[agx_out[:]])
                    # [rank, p, b, h] -> sbuf [p, rank, b, h]; global node
                    # tile nt = rank*8 + b
                    x1c = sb.tile([128, NCORES, 8, F], bf16, tag="x1c")
                    nc.scalar.dma_start(
                        x1c[:], agx_out.rearrange("(r p) b h -> p r b h",
                                                  p=128))

    nc.compile()
    return nc


def _get_nc():
    if "nc" not in _CACHE:
        _CACHE["nc"] = _build_nc()
    return _CACHE["nc"]


def _dropout_mask2():
    """2.0 * bernoulli(key(42), 0.5, (N, F)) exactly as the reference."""
    import jax
    cpu = jax.devices("cpu")[0]
    with jax.default_device(cpu):
        keep = jax.random.bernoulli(jax.random.key(42), 1.0 - DROP_P, (N, F))
        return np.asarray(keep).astype(np.float32) * (1.0 / (1.0 - DROP_P))


def _tile_major(a, nblk):
    """[nblk*128, R] row-major -> [128, nblk, R] so each DMA partition-row
    is one long contiguous read."""
    r = a.shape[1]
    return np.ascontiguousarray(a.reshape(nblk, 128, r).transpose(1, 0, 2))


def prepare_in_maps(xi, x, Ht, Hs, state,
                    w_trans0, theta0, edge_w0, bias0,
                    w_trans1, theta1, edge_w1, bias1,
                    fc_w, fc_b):
    bf = ml_dtypes.bfloat16
    mask2 = _dropout_mask2()

    xbf = _tile_major(np.asarray(x, np.float32).astype(bf), 64)
    fcw32 = np.asarray(fc_w, np.float32)
    fcw_last = float(fcw32[F, 0])
    fcb = float(np.asarray(fc_b, np.float32)[0])

    Hs32 = np.asarray(Hs, np.float32)
    Ht32 = np.asarray(Ht, np.float32)
    xi32 = np.asarray(xi, np.float32)
    st32 = np.asarray(state, np.float32)
    th = [np.asarray(theta0, np.float32), np.asarray(theta1, np.float32)]
    wtr = [np.asarray(w_trans0, np.float32), np.asarray(w_trans1, np.float32)]
    bs = [np.asarray(bias0, np.float32), np.asarray(bias1, np.float32)]
    ews = [np.asarray(edge_w0, np.float32), np.asarray(edge_w1, np.float32)]

    in_maps = []
    for c in range(NCORES):
        e0, e1 = c * E_SH, (c + 1) * E_SH
        n0, n1 = c * N_SH, (c + 1) * N_SH

        pbf = np.zeros((128, PB_W), np.float32)
        pbf[:, PB_TH:PB_TH + 128] = th[0]
        pbf[:, PB_TH + 128:PB_TH + 256] = th[1]
        pbf[:, PB_WT:PB_WT + 128] = wtr[0]
        pbf[:, PB_WT + 128:PB_WT + 256] = wtr[1]
        pbf[:, PB_IDN:PB_IDN + 128] = np.eye(F)
        pbf[:, PB_XIT:PB_XIT + 1024] = xi32[n0:n1, :].T
        pbf[0, PB_B:PB_B + 128] = bs[0]
        pbf[0, PB_B + 128:PB_B + 256] = bs[1]

        pf = np.zeros((128, PF_W), np.float32)
        pf[:, PF_EW:PF_EW + 4] = ews[0][e0:e1].reshape(4, 128).T
        pf[:, PF_EW + 4:PF_EW + 8] = ews[1][e0:e1].reshape(4, 128).T
        pf[:, PF_M2T:PF_M2T + 1024] = mask2[n0:n1, :].T
        pf[:, PF_FCW:PF_FCW + 1] = fcw32[:F, :]
        pf[0, PF_ST:PF_ST + 1024] = st32[n0:n1, 0] * fcw_last + fcb

        in_maps.append({
            "hsT": _tile_major(
                np.ascontiguousarray(Hs32[e0:e1, :].T).astype(bf), 64),
            "ht": _tile_major(
                np.ascontiguousarray(Ht32[:, n0:n1]).astype(bf), 32),
            "xbf": xbf,
            "pbf": pbf.astype(bf),
            "pf32": pf,
        })
    return in_maps


def kernel(xi, x, Ht, Hs, state,
           w_trans0, theta0, edge_w0, bias0,
           w_trans1, theta1, edge_w1, bias1,
           fc_w, fc_b, _trace=False):
    from concourse.bass_utils import run_bass_kernel_spmd

    nc = _get_nc()
    in_maps = prepare_in_maps(
        xi, x, Ht, Hs, state,
        w_trans0, theta0, edge_w0, bias0,
        w_trans1, theta1, edge_w1, bias1,
        fc_w, fc_b)
    res = run_bass_kernel_spmd(
        nc, in_maps, core_ids=list(range(NCORES)), trace=_trace)
    if _trace:
        _CACHE["last_results"] = res
    out = np.concatenate(
        [res.results[c]["out"].reshape(N_SH) for c in range(NCORES)])
    return out.reshape(N, 1).astype(np.float32)


# revision 30
# speedup vs baseline: 1.0868x; 1.0868x over previous
"""Trainium2 Bass kernel for the DQN hypergraph-conv network (8-core SPMD).

Sharding: edges row-sharded for the message stage (Hs@X@theta local per
edge shard), nodes column-sharded for the aggregation stage (Ht.T @ ...),
with AllGather collectives moving the small [E,H]/[N,H] intermediates.
The big Ht/Hs shards are read once in bf16 and stay resident in SBUF
across both conv layers.

Per core c (NCORES=8):
  hsT = Hs[e_c, :].T   [N, E/8]  bf16   (stage-1 moving operand)
  ht  = Ht[:, n_c]     [E, N/8]  bf16   (stage-2 moving operand)
  stage1: tmpT[f,e] = sum_n X[n-tile].T @ hsT[n-tile]      (PE, N=512 free)
  msg[e,h] = tmpT.T @ theta ; scaled = edge_w * msg        -> AllGather
  stage2: aggT[h,n] = w_trans.T @ xiT + bias (rank-1)
          + sum_e scaled[e-tile].T @ ht[e-tile]            (PE, N=512 free)
  epilogue: fused leaky-relu / dropout mask / second lrelu  (DVE)
  conv0 only: PE-transpose X1T -> X1 tiles -> AllGather for conv1
  fc: fc_w.T @ XT (f32 matmul) + host-precomputed state term -> [1, N/8]

Performance notes:
 - Every dma_start costs ~0.6-2us of serial issue time on its engine and
   one InstDMACopy already spans all 16 SDMA engines, so transfers are
   consolidated into a few large DMAs spread over three trigger rings
   (sync, scalar, vector).
 - The big operands are packed TILE-MAJOR on the host ([128, nblk, row])
   so each DMA partition-row is an 8-32KB contiguous read (full HBM BW;
   the natural [N, 128]-style layouts only give 256B rows).
 - Loads are chained with explicit deps so chunks arrive pipelined in
   consumption order instead of fair-sharing bandwidth and all finishing
   together; ht is deferred behind stage 1 to fill the AllGather window.
"""

import sys

if "/opt/trn_rl_repo" not in sys.path:
    sys.path.insert(0, "/opt/trn_rl_repo")

import numpy as np
import ml_dtypes

NCORES = 8
N, E, F = 8192, 4096, 128
E_SH = E // NCORES   # 512 edges per core
N_SH = N // NCORES   # 1024 nodes per core
NEG_SLOPE = 0.01
DROP_P = 0.5

# packed bf16 params layout (columns)
PB_TH = 0          # th0, th1         [128, 128] each
PB_WT = 256        # wt0, wt1
PB_IDN = 512       # identity
PB_XIT = 640       # xiT              [128, 1024]
PB_B = 1664        # b0, b1 on partition 0, 128 cols each
PB_W = 1920
# packed f32 params layout (columns)
PF_EW = 0          # ew0, ew1         [128, 4] each
PF_M2T = 8         # mask2T           [128, 1024]
PF_FCW = 1032      # fc_w[:128]       [128, 1]
PF_ST = 1033       # state term on partition 0, 1024 cols
PF_W = 2060

_CACHE = {}


def _build_nc():
    import concourse.bacc as bacc
    import concourse.mybir as mybir
    import concourse.tile as tile
    from concourse.tile import add_dep_helper

    bf16 = mybir.dt.bfloat16
    f32 = mybir.dt.float32
    Alu = mybir.AluOpType

    nc = bacc.Bacc("TRN2", target_bir_lowering=False, debug=False,
                   num_devices=NCORES)

    hsT_d = nc.dram_tensor("hsT", [128, 64, E_SH], bf16, kind="ExternalInput")
    ht_d = nc.dram_tensor("ht", [128, 32, N_SH], bf16, kind="ExternalInput")
    xbf_d = nc.dram_tensor("xbf", [128, 64, F], bf16, kind="ExternalInput")
    pbf_d = nc.dram_tensor("pbf", [128, PB_W], bf16, kind="ExternalInput")
    pf32_d = nc.dram_tensor("pf32", [128, PF_W], f32, kind="ExternalInput")
    out_d = nc.dram_tensor("out", [1, N_SH], f32, kind="ExternalOutput")

    RG = [list(range(NCORES))]

    with tile.TileContext(nc) as tc:
        with (
            tc.tile_pool(name="sb", bufs=1) as sb,
            tc.tile_pool(name="sc2", bufs=2) as sc2,
            tc.tile_pool(name="ps_tmp", bufs=1, space="PSUM") as ps_tmp,
            tc.tile_pool(name="ps_agg", bufs=2, space="PSUM") as ps_agg,
            tc.tile_pool(name="ps_sm", bufs=2, space="PSUM") as ps_sm,
            tc.tile_pool(name="dram", bufs=1, space="DRAM") as dram,
        ):
            # ---- two pipelined load chains ----
            # sync ring:   hsT chunks 0..3, serially chained so each runs at
            #              full single-DMA bandwidth and arrives in the order
            #              stage 1 consumes it
            # scalar ring: x0, x1, pbf, pf32 (then ht after stage 1)
            hsT_t = [sb.tile([128, 16, E_SH], bf16, tag=f"hsT{i}",
                             name=f"hsT{i}") for i in range(4)]
            xall = [sb.tile([128, 32, F], bf16, tag=f"x{i}", name=f"x{i}")
                    for i in range(2)]
            pbf = sb.tile([128, PB_W], bf16, tag="pbf")
            pf = sb.tile([128, PF_W], f32, tag="pf")

            prev = {"sync": None, "vec": None, "sca": None}

            def chain(ring, engine, dst, src):
                dma = engine.dma_start(dst, src)
                if prev[ring] is not None:
                    add_dep_helper(dma.ins, prev[ring].ins, sync=True,
                                   reason="pipelined load chain")
                prev[ring] = dma
                return dma

            for k in range(4):
                chain("sync", nc.sync, hsT_t[k][:],
                      hsT_d[:, 16 * k:16 * (k + 1), :])
            chain("sca", nc.scalar, xall[0][:], xbf_d[:, 0:32, :])
            chain("sca", nc.scalar, xall[1][:], xbf_d[:, 32:64, :])
            chain("sca", nc.scalar, pbf[:], pbf_d[:, :])
            chain("sca", nc.scalar, pf[:], pf32_d[:, :])

            def th(conv):
                return pbf[:, PB_TH + conv * 128:PB_TH + (conv + 1) * 128]

            def wt(conv):
                return pbf[:, PB_WT + conv * 128:PB_WT + (conv + 1) * 128]

            idn = pbf[:, PB_IDN:PB_IDN + 128]
            xiT = pbf[:, PB_XIT:PB_XIT + 1024]

            def bias(conv):
                return pbf[0:1, PB_B + conv * 128:PB_B + (conv + 1) * 128]

            def ew(conv, ec):
                c0 = PF_EW + conv * 4 + ec
                return pf[:, c0:c0 + 1]

            m2T = pf[:, PF_M2T:PF_M2T + 1024]
            fcw = pf[:, PF_FCW:PF_FCW + 1]
            stT = pf[0:1, PF_ST:PF_ST + 1024]

            ones_sb = sb.tile([1, 512], bf16, tag="ones")
            nc.vector.memset(ones_sb[:], 1.0)

            ht_t = [None, None]

            # collective bounce buffers, PARTITION-MAJOR ([128, blk, F]
            # per rank) so the SBUF<->DRAM bounce DMAs on both sides are
            # contiguous multi-KB rows instead of 256B gathers
            agm_in = [dram.tile([128, 4, F], bf16, tag=f"agmi{i}",
                                name=f"agmi{i}") for i in range(2)]
            agm_out = [dram.tile([NCORES * 128, 4, F], bf16,
                                 addr_space="Shared",
                                 tag=f"agmo{i}", name=f"agmo{i}")
                       for i in range(2)]
            agx_in = dram.tile([128, 8, F], bf16, tag="agxi")
            agx_out = dram.tile([NCORES * 128, 8, F], bf16,
                                addr_space="Shared", tag="agxo")

            x1c = None           # gathered X1 for conv1 stage 1
            xT = [None, None]    # final-layer activations (f32)

            for conv in range(2):
                # ---------- stage 1: tmpT = X.T @ HsT ----------
                tmpT_ps = ps_tmp.tile([128, E_SH], f32, tag="tmpT")
                mm_last = None
                for nt in range(64):
                    lhsT = (xall[nt // 32][:, nt % 32, :] if conv == 0
                            else x1c[:, nt // 8, nt % 8, :])
                    mm_last = nc.tensor.matmul(
                        tmpT_ps[:], lhsT, hsT_t[nt // 16][:, nt % 16, :],
                        start=(nt == 0), stop=(nt == 63))

                if conv == 0:
                    # ht loads deferred behind stage 1: they fill the
                    # AllGather window instead of stealing stage-1 BW.
                    for i in range(2):
                        hc = sb.tile([128, 16, N_SH], bf16, tag=f"ht{i}",
                                     name=f"ht{i}")
                        dma = nc.scalar.dma_start(
                            hc[:], ht_d[:, i * 16:(i + 1) * 16, :])
                        add_dep_helper(dma.ins, mm_last.ins, sync=True,
                                       reason="defer ht behind stage1")
                        ht_t[i] = hc

                tmpT_bf = sb.tile([128, E_SH], bf16, tag=f"tmpTbf{conv}")
                nc.vector.tensor_copy(tmpT_bf[:], tmpT_ps[:])

                # ---------- msg = tmpT.T @ theta, scaled by edge_w ----------
                msg_sb = sb.tile([128, 4, F], bf16, tag="msg")
                for ec in range(4):
                    mps = ps_sm.tile([128, F], f32, tag="msg", bufs=2)
                    nc.tensor.matmul(
                        mps[:], tmpT_bf[:, ec * 128:(ec + 1) * 128],
                        th(conv), start=True, stop=True)
                    nc.vector.tensor_scalar(
                        msg_sb[:, ec, :], mps[:], ew(conv, ec), None, Alu.mult)
                nc.sync.dma_start(agm_in[conv][:], msg_sb[:])

                nc.gpsimd.collective_compute(
                    "AllGather", Alu.bypass, replica_groups=RG,
                    ins=[agm_in[conv][:]], outs=[agm_out[conv][:]])

                # [rank, p, c, h] -> sbuf [p, rank, c, h]; global edge tile
                # et = rank*4 + c.  Two chained half-reads so stage 2 can
                # start on ranks 0-3 while ranks 4-7 are still landing.
                sc_t = sb.tile([128, NCORES, 4, F], bf16, tag="sc")
                agm_r = agm_out[conv].rearrange("(r p) c h -> p r c h", p=128)
                d1 = nc.scalar.dma_start(sc_t[:, 0:4, :, :],
                                         agm_r[:, 0:4, :, :])
                d2 = nc.scalar.dma_start(sc_t[:, 4:8, :, :],
                                         agm_r[:, 4:8, :, :])
                add_dep_helper(d2.ins, d1.ins, sync=True,
                               reason="pipelined sc readback")

                # ---------- stage 2: aggT = wT@xiT + b + scaled.T @ Ht ----------
                for nb in range(2):
                    agg = ps_agg.tile([128, 512], f32, tag="agg")
                    nc.tensor.matmul(
                        agg[:], wt(conv), xiT[:, nb * 512:(nb + 1) * 512],
                        start=True, stop=False)
                    nc.tensor.matmul(
                        agg[:], bias(conv), ones_sb[:],
                        start=False, stop=False)
                    for et in range(32):
                        nc.tensor.matmul(
                            agg[:], sc_t[:, et // 4, et % 4, :],
                            ht_t[et // 16][:, et % 16,
                                           nb * 512:(nb + 1) * 512],
                            start=False, stop=(et == 31))

                    if conv == 0:
                        # X1T = lrelu(agg) * dropout_mask, transpose to
                        # node-major, bounce out (per nb, pipelined)
                        sl = sc2.tile([128, 512], f32, tag="sl")
                        nc.vector.tensor_scalar(
                            sl[:], agg[:], NEG_SLOPE, None, Alu.mult)
                        lr = sc2.tile([128, 512], f32, tag="lr")
                        nc.vector.tensor_tensor(lr[:], agg[:], sl[:], Alu.max)
                        x1t = sb.tile([128, 512], bf16, tag=f"x1t{nb}")
                        nc.vector.tensor_tensor(
                            x1t[:], lr[:], m2T[:, nb * 512:(nb + 1) * 512],
                            Alu.mult)

                        x1loc = sb.tile([128, 4, F], bf16, tag=f"x1loc{nb}",
                                        name=f"x1loc{nb}")
                        for t in range(4):
                            tps = ps_sm.tile([128, 128], bf16, tag="tr",
                                             bufs=2)
                            nc.tensor.transpose(
                                tps[:], x1t[:, t * 128:(t + 1) * 128], idn)
                            nc.vector.tensor_copy(x1loc[:, t, :], tps[:])
                        nc.sync.dma_start(
                            agx_in[:, nb * 4:(nb + 1) * 4, :], x1loc[:])
                    else:
                        # X = lrelu(lrelu(agg)) = max(agg, 1e-4*agg)  (f32)
                        sl = sc2.tile([128, 512], f32, tag="sl")
                        nc.vector.tensor_scalar(
                            sl[:], agg[:], NEG_SLOPE * NEG_SLOPE, None,
                            Alu.mult)
                        t = sb.tile([128, 512], f32, tag=f"xT{nb}")
                        nc.vector.tensor_tensor(t[:], agg[:], sl[:], Alu.max)
                        xT[nb] = t
                        # fc for this block immediately
                        fps = ps_sm.tile([1, 512], f32, tag="fc", bufs=1)
                        nc.tensor.matmul(fps[:], fcw, t[:],
                                         start=True, stop=True)
                        osb = sc2.tile([1, 512], f32, tag="osb")
                        nc.vector.tensor_tensor(
                            osb[:], fps[:], stT[:, nb * 512:(nb + 1) * 512],
                            Alu.add)
                        nc.sync.dma_start(
                            out_d[0:1, nb * 512:(nb + 1) * 512], osb[:])

                if conv == 0:
                    nc.gpsimd.collective_compute(
                        "AllGather", Alu.bypass, replica_groups=RG,
                        ins=[agx_in[:]], outs=[agx_out[:]])
                    # [rank, p, b, h] -> sbuf [p, rank, b, h]; global node
                    # tile nt = rank*8 + b.  Chained half-reads as above.
                    x1c = sb.tile([128, NCORES, 8, F], bf16, tag="x1c")
                    agx_r = agx_out.rearrange("(r p) b h -> p r b h", p=128)
                    d1 = nc.scalar.dma_start(x1c[:, 0:4, :, :],
                                             agx_r[:, 0:4, :, :])
                    d2 = nc.scalar.dma_start(x1c[:, 4:8, :, :],
                                             agx_r[:, 4:8, :, :])
                    add_dep_helper(d2.ins, d1.ins, sync=True,
                                   reason="pipelined x1 readback")

    nc.compile()
    return nc


def _get_nc():
    if "nc" not in _CACHE:
        _CACHE["nc"] = _build_nc()
    return _CACHE["nc"]


def _dropout_mask2():
    """2.0 * bernoulli(key(42), 0.5, (N, F)) exactly as the reference."""
    import jax
    cpu = jax.devices("cpu")[0]
    with jax.default_device(cpu):
        keep = jax.random.bernoulli(jax.random.key(42), 1.0 - DROP_P, (N, F))
        return np.asarray(keep).astype(np.float32) * (1.0 / (1.0 - DROP_P))


def _tile_major(a, nblk):
    """[nblk*128, R] row-major -> [128, nblk, R] so each DMA partition-row
    is one long contiguous read."""
    r = a.shape[1]
    return np.ascontiguousarray(a.reshape(nblk, 128, r).transpose(1, 0, 2))


def prepare_in_maps(xi, x, Ht, Hs, state,
                    w_trans0, theta0, edge_w0, bias0,
                    w_trans1, theta1, edge_w1, bias1,
                    fc_w, fc_b):
    bf = ml_dtypes.bfloat16
    mask2 = _dropout_mask2()

    xbf = _tile_major(np.asarray(x, np.float32).astype(bf), 64)
    fcw32 = np.asarray(fc_w, np.float32)
    fcw_last = float(fcw32[F, 0])
    fcb = float(np.asarray(fc_b, np.float32)[0])

    Hs32 = np.asarray(Hs, np.float32)
    Ht32 = np.asarray(Ht, np.float32)
    xi32 = np.asarray(xi, np.float32)
    st32 = np.asarray(state, np.float32)
    th = [np.asarray(theta0, np.float32), np.asarray(theta1, np.float32)]
    wtr = [np.asarray(w_trans0, np.float32), np.asarray(w_trans1, np.float32)]
    bs = [np.asarray(bias0, np.float32), np.asarray(bias1, np.float32)]
    ews = [np.asarray(edge_w0, np.float32), np.asarray(edge_w1, np.float32)]

    in_maps = []
    for c in range(NCORES):
        e0, e1 = c * E_SH, (c + 1) * E_SH
        n0, n1 = c * N_SH, (c + 1) * N_SH

        pbf = np.zeros((128, PB_W), np.float32)
        pbf[:, PB_TH:PB_TH + 128] = th[0]
        pbf[:, PB_TH + 128:PB_TH + 256] = th[1]
        pbf[:, PB_WT:PB_WT + 128] = wtr[0]
        pbf[:, PB_WT + 128:PB_WT + 256] = wtr[1]
        pbf[:, PB_IDN:PB_IDN + 128] = np.eye(F)
        pbf[:, PB_XIT:PB_XIT + 1024] = xi32[n0:n1, :].T
        pbf[0, PB_B:PB_B + 128] = bs[0]
        pbf[0, PB_B + 128:PB_B + 256] = bs[1]

        pf = np.zeros((128, PF_W), np.float32)
        pf[:, PF_EW:PF_EW + 4] = ews[0][e0:e1].reshape(4, 128).T
        pf[:, PF_EW + 4:PF_EW + 8] = ews[1][e0:e1].reshape(4, 128).T
        pf[:, PF_M2T:PF_M2T + 1024] = mask2[n0:n1, :].T
        pf[:, PF_FCW:PF_FCW + 1] = fcw32[:F, :]
        pf[0, PF_ST:PF_ST + 1024] = st32[n0:n1, 0] * fcw_last + fcb

        in_maps.append({
            "hsT": _tile_major(
                np.ascontiguousarray(Hs32[e0:e1, :].T).astype(bf), 64),
            "ht": _tile_major(
                np.ascontiguousarray(Ht32[:, n0:n1]).astype(bf), 32),
            "xbf": xbf,
            "pbf": pbf.astype(bf),
            "pf32": pf,
        })
    return in_maps


def kernel(xi, x, Ht, Hs, state,
           w_trans0, theta0, edge_w0, bias0,
           w_trans1, theta1, edge_w1, bias1,
           fc_w, fc_b, _trace=False):
    from concourse.bass_utils import run_bass_kernel_spmd

    nc = _get_nc()
    in_maps = prepare_in_maps(
        xi, x, Ht, Hs, state,
        w_trans0, theta0, edge_w0, bias0,
        w_trans1, theta1, edge_w1, bias1,
        fc_w, fc_b)
    res = run_bass_kernel_spmd(
        nc, in_maps, core_ids=list(range(NCORES)), trace=_trace)
    if _trace:
        _CACHE["last_results"] = res
    out = np.concatenate(
        [res.results[c]["out"].reshape(N_SH) for c in range(NCORES)])
    return out.reshape(N, 1).astype(np.float32)
